# revision 18
# baseline (speedup 1.0000x reference)
"""Trainium2 Bass kernel for nn_CoreferenceResolver (coref UNet + pair decoder).

Sharding: core c handles batch b=c//2 and pair-half h=c%2 (496 of 992 pairs).
The gather/cosine/UNet stages are replicated on the two cores sharing a batch;
the extractor linears and group-bilinear decoder are sharded over pairs.

v1 design notes (vs the f32r baseline):
- all weights/activations bf16 (halves DMA bytes, PE still 1 cycle/row)
- fin 1x1 conv + amap gathers + W2 matmuls folded into host-precomputed
  W2' = fin_w^T @ head_w[768:] and a single d1 gather
- extractor = one stacked K=128 matmul per (k, extractor):
  mov rows 0:64 d1[pairs], 64:96 one-hot(hi) (host), 96:128 one-hot(ti)
- enc1 via 3 column-shifted padded images (K=3 matmuls, 6 total)
- weights arrive as a few packed DMA chunks ordered just-in-time so the
  cos->image DMA never queues behind megabytes of weight traffic
- PE warmup chain holds the p-state ramp so real matmuls price at full speed
"""
import os
import sys

for _p in ("/opt/trn_rl_repo",):
    if os.path.isdir(_p) and _p not in sys.path:
        sys.path.insert(0, _p)

import numpy as np

import concourse.bass as bass
import concourse.tile as tile
from concourse import bacc, mybir
from concourse.bass_utils import run_bass_kernel_spmd

f32 = mybir.dt.float32
i16 = mybir.dt.int16
bf16 = mybir.dt.bfloat16
AF = mybir.ActivationFunctionType
OP = mybir.AluOpType

B, L, D, H = 4, 1024, 768, 12
NE, P = 32, 992
BLOCK = 64
G = D // BLOCK          # 12 groups
OUT_CH = 256
NCORES = 8
NH = P // 2             # 496 pairs per core
KD = D // 128           # 6 chunks of the D dim

# ---------------------------------------------------------------------------
# packed-chunk layouts (shapes only; shared by build_nc and pack_inputs)
# entries: name -> (row0, rows, col0, cols); each chunk = one DRAM tensor.
# ---------------------------------------------------------------------------


def _mklayout(rows, entries):
    lay, col = {}, 0
    for name, r0, r, c in entries:
        lay[name] = (r0, r, col, c)
        col += c
    return lay, col


LAY_A64, NC_A64 = _mklayout(64, [
    ("ident", 0, 32, 32),
    ("enc1w3", 0, 3, 192),          # [dx, dy*64+c]
    ("enc2w", 0, 64, 1152),         # [64, 9*128]
    ("dec1wb", 0, 64, 576),         # [64, 9*64]
    ("ag1wx", 0, 64, 64),
    ("ag1ps", 0, 64, 1),
])
LAY_BOTT, NC_BOTT = _mklayout(128, [("bott", 0, 128, 2304)])   # [128, 9*256]
LAY_AG2, NC_AG2 = _mklayout(128, [
    ("ag2wg", 0, 128, 256),         # [128, 2*128]
    ("ag2wx", 0, 128, 128),
    ("ag2ps", 0, 128, 1),
])
LAY_DEC2A, NC_DEC2A = _mklayout(128, [("dec2a", 0, 128, 2304)])  # kc 0:2
LAY_DEC2B, NC_DEC2B = _mklayout(128, [("dec2b", 0, 128, 1152)])  # kc 2
LAY_B128, NC_B128 = _mklayout(128, [
    ("dec1wa", 0, 128, 576),        # [128, 9*64]
    ("ag1wg", 0, 128, 64),
])
LAY_F, NC_F = _mklayout(128, [
    ("stk_h", 0, 128, 768),         # rows 0:64 W2h'; 64:96 <- EW1 (device)
    ("stk_t", 0, 128, 768),         # rows 0:64 W2t'; 64:96 <- EW1t (device)
    ("wdecA", 0, 128, 768),         # blockdiag per k-chunk, o=0
    ("wdecB", 0, 128, 768),         # blockdiag per k-chunk, o=1
    ("ssum", 0, 128, 4),            # [all-ones|0 ; 0|all-ones] selectors
])
LAY_MF, NC_MF = _mklayout(128, [
    ("emask", 0, 32, 1),
    ("enc1b", 0, 64, 1),
    ("enc2b", 0, 128, 1),
    ("bottb", 0, 128, 2),
    ("dec2b_b", 0, 128, 1),
    ("dec1b", 0, 64, 1),
    ("hbp", 0, 128, 6),
    ("tbp", 0, 128, 6),
    ("decb", 0, 2, 1),
])


def build_nc():
    nc = bacc.Bacc("TRN2", target_bir_lowering=False, debug=False, num_devices=NCORES)

    def inp(name, shape, dt=f32):
        return nc.dram_tensor(name, shape, dt, kind="ExternalInput")

    x_b = inp("x_b", [L, D])
    eidx_d = inp("eidx", [128, 8], i16)
    pidx_d = inp("pidx", [128, NH // 16], i16)
    ohm_d = inp("ohm", [64, NH], bf16)
    mf_d = inp("mf", [128, NC_MF])
    a64_d = inp("a64", [64, NC_A64], bf16)
    bott_d = inp("bott_c", [128, NC_BOTT], bf16)
    ag2_d = inp("ag2_c", [128, NC_AG2], bf16)
    dec2a_d = inp("dec2a_c", [128, NC_DEC2A], bf16)
    dec2b_d = inp("dec2b_c", [128, NC_DEC2B], bf16)
    b128_d = inp("b128", [128, NC_B128], bf16)
    w1h_d = inp("w1h", [128, KD * D], bf16)
    w1t_d = inp("w1t", [128, KD * D], bf16)
    f_d = inp("f_c", [128, NC_F], bf16)

    y = nc.dram_tensor("y", [2, NH], f32, kind="ExternalOutput")

    from contextlib import ExitStack
    with tile.TileContext(nc) as tc, ExitStack() as _ctx:
        sbw = _ctx.enter_context(tc.tile_pool(name="sbw", bufs=1))   # persistent
        sbt = _ctx.enter_context(tc.tile_pool(name="sbt", bufs=3))   # rotating temps
        pu_cm = tc.tile_pool(name="pu", bufs=2, space="PSUM")
        pu = pu_cm.__enter__()

        # ---------------- t0: gpsimd: eidx, gather, warm tile, memsets -----
        t_eidx = sbw.tile([128, 8], i16, tag="eidx")
        nc.sync.dma_start(t_eidx[:], eidx_d[:])
        warm = sbw.tile([1, 512], bf16, tag="warm")
        nc.vector.memset(warm[:], 0.0)
        # entities replicated on partition blocks 0:32 / 32:64 / 64:96 / 96:128
        ent_raw = sbw.tile([128, 1, D], f32, tag="entraw")
        nc.gpsimd.dma_gather(ent_raw[:], x_b[:], t_eidx[:],
                             num_idxs=128, num_idxs_reg=128, elem_size=D)
        ent = ent_raw[0:NE, 0, :]

        # padded intermediates (bf16) + border-only memsets
        img3 = sbw.tile([3, 34 * 34], bf16, tag="img3")
        c1p = sbw.tile([64, 34 * 34], bf16, tag="c1p")
        p1p = sbw.tile([64, 18 * 18], bf16, tag="p1p")
        c2p = sbw.tile([128, 18 * 18], bf16, tag="c2p")
        p2p = sbw.tile([128, 10 * 10], bf16, tag="p2p")
        u2p0 = sbw.tile([128, 18 * 18], bf16, tag="u2p0")
        u2p1 = sbw.tile([128, 18 * 18], bf16, tag="u2p1")
        att2p = sbw.tile([128, 18 * 18], bf16, tag="att2p")
        u1p = sbw.tile([128, 34 * 34], bf16, tag="u1p")
        att1p = sbw.tile([64, 34 * 34], bf16, tag="att1p")

        ones_bf = sbw.tile([1, 128], bf16, tag="ones")
        nc.vector.memset(ones_bf[:], 1.0)
        img3v0 = img3[:].rearrange("c (h w) -> c h w", h=34, w=34)
        nc.vector.memset(img3v0[:, 0:34:33, :], 0.0)
        nc.vector.memset(img3v0[:, :, 0:2], 0.0)
        nc.vector.memset(img3v0[:, :, 32:34], 0.0)

        def borders(t, n):
            v = t[:].rearrange("c (h w) -> c h w", h=n, w=n)
            nc.vector.memset(v[:, 0:n:n - 1, :], 0.0)
            nc.vector.memset(v[:, :, 0:n:n - 1], 0.0)

        for t, n in ((c1p, 34), (p1p, 18), (c2p, 18), (p2p, 10), (u2p0, 18),
                     (u2p1, 18), (att2p, 18), (u1p, 34), (att1p, 34)):
            borders(t, n)

        # ---------------- sync-engine DMA chunks (just-in-time order) ------
        def load(dram, shape, dt, tag, eng=None):
            t = sbw.tile(shape, dt, tag=tag)
            (eng or nc.sync).dma_start(t[:], dram[:])
            return t

        t_mf = load(mf_d, [128, NC_MF], f32, "mf")
        t_a64 = load(a64_d, [64, NC_A64], bf16, "a64")

        def loadE(dram, shape, dt, tag):
            t = sbw.tile(shape, dt, tag=tag)
            nc.vector.tensor_copy(t[0:1, 0:1], ent_raw[0:1, 0, 0:1])
            nc.sync.dma_start(t[:], dram[:])
            return t

        t_bott = loadE(bott_d, [128, NC_BOTT], bf16, "bott")
        t_ag2 = loadE(ag2_d, [128, NC_AG2], bf16, "ag2")
        t_pidx = load(pidx_d, [128, NH // 16], i16, "pidx")

        mov_h = sbw.tile([96, NH], bf16, tag="movh")
        mov_t = sbw.tile([96, NH], bf16, tag="movt")
        nc.sync.dma_start(mov_h[64:96, :], ohm_d[0:32, :])
        nc.sync.dma_start(mov_t[64:96, :], ohm_d[32:64, :])

        def vw(tile_, lay, name, shape=None):
            row0, r, c0, c = lay[name]
            ap = tile_[row0:row0 + r, c0:c0 + c]
            if shape is not None and len(shape) > 2:
                pat = {3: "p (a b) -> p a b", 4: "p (a b c) -> p a b c"}[len(shape)]
                kw = dict(zip("abc", shape[1:]))
                ap = ap.rearrange(pat, **kw)
            return ap

        t_ident = vw(t_a64, LAY_A64, "ident")
        t_enc1w = vw(t_a64, LAY_A64, "enc1w3", (3, 3, 64))
        t_enc2w = vw(t_a64, LAY_A64, "enc2w", (64, 9, 128))
        t_dec1wb = vw(t_a64, LAY_A64, "dec1wb", (64, 9, 64))
        t_ag1wx = vw(t_a64, LAY_A64, "ag1wx")
        t_ag1ps = vw(t_a64, LAY_A64, "ag1ps")
        t_bottw = vw(t_bott, LAY_BOTT, "bott", (128, 9, 256))
        t_ag2wg = vw(t_ag2, LAY_AG2, "ag2wg", (128, 2, 128))
        t_ag2wx = vw(t_ag2, LAY_AG2, "ag2wx")
        t_ag2ps = vw(t_ag2, LAY_AG2, "ag2ps")

        t_emask = vw(t_mf, LAY_MF, "emask")
        t_enc1b = vw(t_mf, LAY_MF, "enc1b")
        t_enc2b = vw(t_mf, LAY_MF, "enc2b")
        t_bottb = vw(t_mf, LAY_MF, "bottb")
        t_dec2bb = vw(t_mf, LAY_MF, "dec2b_b")
        t_dec1b = vw(t_mf, LAY_MF, "dec1b")
        t_hbp = vw(t_mf, LAY_MF, "hbp")
        t_tbp = vw(t_mf, LAY_MF, "tbp")
        t_decb = vw(t_mf, LAY_MF, "decb")

        # ---------------- PE warmup chain (p-state ramp) -------------------
        # keeps one unbroken PE busy-run so later bursts price at full speed
        pw_cm = tc.tile_pool(name="pw", bufs=1, space="PSUM")
        pw = pw_cm.__enter__()
        p_warm = pw.tile([1, 512], f32, tag="pw")

        def filler(n):
            for _ in range(n):
                nc.tensor.matmul(p_warm[:], warm[0:1, 0:1], warm[:],
                                 start=True, stop=True)

        filler(int(os.environ.get("NWARM", "14")))

        # ---------------- front-end: norms + transposes + cos --------------
        # dummy sqrt at t0 -> the preamble table load covers Sqrt+Square
        dummy = sbw.tile([1, 2], f32, tag="dum")
        nc.scalar.activation(dummy[:, 0:1], warm[0:1, 0:1], AF.Sqrt)
        sq_scr = sbt.tile([128, D], bf16, tag="t")
        ss = sbw.tile([128, 1], f32, tag="ss")
        nc.scalar.activation(sq_scr[0:96, :], ent_raw[0:96, 0, :], AF.Square,
                             accum_out=ss[0:96, :])
        normc = sbw.tile([128, 1], f32, tag="normc")
        nc.scalar.activation(normc[0:96, :], ss[0:96, :], AF.Sqrt)
        # dummy sigmoid -> hoist the 2nd act-table load off the critical path
        nc.scalar.activation(dummy[:, 1:2], ss[0:1, :], AF.Sigmoid)
        nc.vector.tensor_single_scalar(normc[0:96, :], normc[0:96, :], 1e-13, op=OP.max)
        rinv = sbw.tile([NE, 1], f32, tag="rinv")
        nc.vector.reciprocal(rinv[:], normc[0:NE, :])
        nc.vector.tensor_tensor(out=rinv[:], in0=rinv[:], in1=t_emask, op=OP.mult)
        nrm = sbw.tile([NE, D], bf16, tag="nrm")
        nc.vector.tensor_scalar(out=nrm[:], in0=ent, scalar1=rinv[:],
                                scalar2=None, op0=OP.mult)

        nrmT = sbw.tile([128, KD, NE], bf16, tag="nrmT")
        p_T = pu.tile([128, KD * NE], bf16, tag="pu")
        for k in range(KD):
            nc.tensor.transpose(p_T[:, k * NE:(k + 1) * NE],
                                nrm[:, k * 128:(k + 1) * 128], t_ident)
        nc.vector.tensor_copy(nrmT[:], p_T[:].rearrange("p (a b) -> p a b", a=KD))

        p_cos = pu.tile([NE, NE], f32, tag="pu")
        for k in range(KD):
            nc.tensor.matmul(p_cos[:], nrmT[:, k, :], nrmT[:, k, :],
                             start=(k == 0), stop=(k == KD - 1))
        s_cos = sbw.tile([NE, NE], bf16, tag="scos")
        nc.vector.tensor_copy(s_cos[:], p_cos[:])
        filler(int(os.environ.get("NFILL1", "4")))

        # ---------------- image staging: 3 column-shifted padded copies ----
        img3v = img3[:].rearrange("c (h w) -> c h w", h=34, w=34)
        nc.sync.dma_start(img3v[0:1, 1:33, 2:34], s_cos[:])
        nc.scalar.dma_start(img3v[1:2, 1:33, 1:33], s_cos[:])
        nc.gpsimd.dma_start(img3v[2:3, 1:33, 0:32], s_cos[:])

        # remaining weight chunks: a tiny token copy (reads s_cos) makes each
        # chunk DMA wait until the front-end is done with the DMA device
        def loadT(dram, shape, dt, tag):
            t = sbw.tile(shape, dt, tag=tag)
            nc.vector.tensor_copy(t[0:1, 0:1], s_cos[0:1, 0:1])
            nc.sync.dma_start(t[:], dram[:])
            return t

        t_w1h = loadT(w1h_d, [128, KD * D], bf16, "w1h")
        t_w1h = t_w1h[:].rearrange("p (k d) -> p k d", k=KD)
        t_w1t = loadT(w1t_d, [128, KD * D], bf16, "w1t")
        t_w1t = t_w1t[:].rearrange("p (k d) -> p k d", k=KD)
        t_dec2wa = loadT(dec2a_d, [128, NC_DEC2A], bf16, "dec2a")
        t_dec2wa = t_dec2wa[:].rearrange("p (a b c) -> p a b c", a=2, b=9, c=128)
        t_dec2wb = loadT(dec2b_d, [128, NC_DEC2B], bf16, "dec2b")
        t_dec2wb = t_dec2wb[:].rearrange("p (b c) -> p b c", b=9, c=128)
        t_b128 = loadT(b128_d, [128, NC_B128], bf16, "b128")
        t_dec1wa = vw(t_b128, LAY_B128, "dec1wa", (128, 9, 64))
        t_ag1wg = vw(t_b128, LAY_B128, "ag1wg")
        t_f = loadT(f_d, [128, NC_F], bf16, "f")
        stk_h = vw(t_f, LAY_F, "stk_h", (128, KD, 128))
        stk_t = vw(t_f, LAY_F, "stk_t", (128, KD, 128))
        t_wdecA = vw(t_f, LAY_F, "wdecA", (128, KD, 128))
        t_wdecB = vw(t_f, LAY_F, "wdecB", (128, KD, 128))
        t_ssum = vw(t_f, LAY_F, "ssum")

        # ---------------- enc1: 2 halves x 3 dy matmuls (K=3) --------------
        c1pv = c1p[:].rearrange("c (h w) -> c h w", h=34, w=34)
        p_c1 = pu.tile([64, 1024], f32, tag="pu")
        for hh in range(2):
            for dy in range(3):
                rows = slice(dy + 16 * hh, dy + 16 * hh + 16)
                nc.tensor.matmul(p_c1[:, hh * 512:(hh + 1) * 512],
                                 t_enc1w[:, dy, :],
                                 img3v[0:3, rows, 1:33],
                                 start=(dy == 0), stop=(dy == 2))
        filler(int(os.environ.get("NFILL2", "7")))
        # maxpool commutes with relu+per-channel-bias: pool PSUM directly
        p1pv = p1p[:].rearrange("c (h w) -> c h w", h=18, w=18)
        p_c1v = p_c1[:].rearrange("c (h w) -> c h w", h=32, w=32)
        tmp = sbt.tile([64, 16, 16], f32, tag="t")
        nc.vector.tensor_max(tmp[:], p_c1v[:, 0:32:2, 0:32:2], p_c1v[:, 0:32:2, 1:32:2])
        nc.vector.tensor_max(tmp[:], tmp[:], p_c1v[:, 1:32:2, 0:32:2])
        nc.vector.tensor_max(tmp[:], tmp[:], p_c1v[:, 1:32:2, 1:32:2])
        nc.scalar.activation(p1pv[:, 1:17, 1:17], tmp[:], AF.Relu, bias=t_enc1b)
        nc.scalar.activation(c1pv[:, 1:33, 1:33], p_c1v[:],
                             AF.Relu, bias=t_enc1b)

        # ---------------- enc2: 9 taps K=64 --------------------------------
        p_c2 = pu.tile([128, 256], f32, tag="pu")
        for tap in range(9):
            dy, dx = tap // 3, tap % 3
            nc.tensor.matmul(p_c2[:], t_enc2w[:, tap, :],
                             p1pv[:, dy:dy + 16, dx:dx + 16],
                             start=(tap == 0), stop=(tap == 8))
        filler(int(os.environ.get("NFILL3", "4")))
        c2pv = c2p[:].rearrange("c (h w) -> c h w", h=18, w=18)
        p2pv = p2p[:].rearrange("c (h w) -> c h w", h=10, w=10)
        p_c2v = p_c2[:].rearrange("c (h w) -> c h w", h=16, w=16)
        tmp2 = sbt.tile([128, 8, 8], f32, tag="t")
        nc.vector.tensor_max(tmp2[:], p_c2v[:, 0:16:2, 0:16:2], p_c2v[:, 0:16:2, 1:16:2])
        nc.vector.tensor_max(tmp2[:], tmp2[:], p_c2v[:, 1:16:2, 0:16:2])
        nc.vector.tensor_max(tmp2[:], tmp2[:], p_c2v[:, 1:16:2, 1:16:2])
        nc.scalar.activation(p2pv[:, 1:9, 1:9], tmp2[:], AF.Relu, bias=t_enc2b)
        nc.scalar.activation(c2pv[:, 1:17, 1:17], p_c2v[:],
                             AF.Relu, bias=t_enc2b)

        # ---------------- EW-head premultiply (fills the pool stalls) ------
        ew_cm = tc.tile_pool(name="ew", bufs=1, space="PSUM")
        ew = ew_cm.__enter__()
        p_ewh = ew.tile([128, D], f32, tag="ew")
        for k in range(KD):
            for n0, n1 in ((0, 512), (512, D)):
                nc.tensor.matmul(p_ewh[64:96, n0:n1], nrmT[:, k, :],
                                 t_w1h[:, k, n0:n1],
                                 start=(k == 0), stop=(k == KD - 1))

        # EW-head PSUM -> stacked weights (frees the shared EW psum tile)
        nc.scalar.activation(stk_h[64:96, :, :].rearrange("p a b -> p (a b)"),
                             p_ewh[64:96, :], AF.Copy, scale=normc[64:96, :])

        # ---------------- bottleneck: 9 taps x 2 M-chunks, K=128 -----------
        c3 = []
        for mc in range(2):
            p_c3 = pu.tile([128, 64], f32, tag="pu")
            for tap in range(9):
                dy, dx = tap // 3, tap % 3
                nc.tensor.matmul(p_c3[:], t_bottw[:, tap, mc * 128:(mc + 1) * 128],
                                 p2pv[:, dy:dy + 8, dx:dx + 8],
                                 start=(tap == 0), stop=(tap == 8))
            c3s = sbt.tile([128, 8, 8], bf16, tag=f"c3_{mc}")
            nc.scalar.activation(c3s[:], p_c3[:].rearrange("c (h w) -> c h w", h=8, w=8),
                                 AF.Relu, bias=t_bottb[:, mc:mc + 1])
            c3.append(c3s)

        # ---------------- up2 ----------------------------------------------
        u2p0v = u2p0[:].rearrange("c (h w) -> c h w", h=18, w=18)
        u2p1v = u2p1[:].rearrange("c (h w) -> c h w", h=18, w=18)
        for src, dv in ((c3[0], u2p0v), (c3[1], u2p1v)):
            for i in range(2):
                for j in range(2):
                    nc.vector.tensor_copy(dv[:, 1 + i:17:2, 1 + j:17:2], src[:])

        # ---------------- attention gate 2 + dec2 (interleaved) ------------
        # the 18 u2-taps of dec2 fill the PE while the psi/sigmoid chain of
        # the attention gate bounces between ACT and DVE
        p_a2 = pu.tile([128, 256], f32, tag="pu")
        nc.tensor.matmul(p_a2[:], t_ag2wg[:, 0, :], u2p0v[:, 1:17, 1:17],
                         start=True, stop=False)
        nc.tensor.matmul(p_a2[:], t_ag2wg[:, 1, :], u2p1v[:, 1:17, 1:17],
                         start=False, stop=False)
        nc.tensor.matmul(p_a2[:], t_ag2wx, c2pv[:, 1:17, 1:17],
                         start=False, stop=True)
        p_d2 = pu.tile([128, 256], f32, tag="pu")
        n_mm = 0
        for kc in range(2):
            src = (u2p0v, u2p1v)[kc]
            for tap in range(9):
                dy, dx = tap // 3, tap % 3
                nc.tensor.matmul(p_d2[:], t_dec2wa[:, kc, tap, :],
                                 src[:, dy:dy + 16, dx:dx + 16],
                                 start=(n_mm == 0), stop=False)
                n_mm += 1
        p_ewt = ew.tile([128, D], f32, tag="ew")
        for k in range(KD):
            for n0, n1 in ((0, 512), (512, D)):
                nc.tensor.matmul(p_ewt[64:96, n0:n1], nrmT[:, k, :],
                                 t_w1t[:, k, n0:n1],
                                 start=(k == 0), stop=(k == KD - 1))
        r2 = sbt.tile([128, 256], bf16, tag="t")
        nc.scalar.activation(r2[:], p_a2[:], AF.Relu)
        p_g2 = pu.tile([1, 256], f32, tag="pu")
        nc.tensor.matmul(p_g2[:], t_ag2ps, r2[:])
        a2 = sbt.tile([1, 256], bf16, tag="a2")
        nc.scalar.activation(a2[:], p_g2[:], AF.Sigmoid)
        p_a2b = pu.tile([128, 256], f32, tag="pu")
        nc.tensor.matmul(p_a2b[:], ones_bf[:], a2[:])
        att2pv = att2p[:].rearrange("c (h w) -> c h w", h=18, w=18)
        nc.vector.tensor_mul(att2pv[:, 1:17, 1:17],
                             p_a2b[:].rearrange("c (h w) -> c h w", h=16, w=16),
                             c2pv[:, 1:17, 1:17])
        for tap in range(9):
            dy, dx = tap // 3, tap % 3
            nc.tensor.matmul(p_d2[:], t_dec2wb[:, tap, :],
                             att2pv[:, dy:dy + 16, dx:dx + 16],
                             start=False, stop=(tap == 8))
        d2s = sbt.tile([128, 256], bf16, tag="d2s")
        nc.scalar.activation(d2s[:], p_d2[:], AF.Relu, bias=t_dec2bb)
        nc.scalar.activation(stk_t[64:96, :, :].rearrange("p a b -> p (a b)"),
                             p_ewt[64:96, :], AF.Copy, scale=normc[64:96, :])

        # ---------------- up1 ----------------------------------------------
        u1pv = u1p[:].rearrange("c (h w) -> c h w", h=34, w=34)
        d2v = d2s[:].rearrange("c (h w) -> c h w", h=16, w=16)
        for i in range(2):
            for j in range(2):
                nc.vector.tensor_copy(u1pv[:, 1 + i:33:2, 1 + j:33:2], d2v[:])

        # ---------------- attention gate 1 + dec1 + EW (interleaved) -------
        # dec1's u1-taps and the EW premultiplies fill the PE while the
        # psi/sigmoid chain runs; att1-taps close the dec1 groups afterwards
        p_a1 = pu.tile([64, 1024], f32, tag="pu")
        for hh in range(2):
            rows = slice(1 + 16 * hh, 17 + 16 * hh)
            nc.tensor.matmul(p_a1[:, hh * 512:(hh + 1) * 512], t_ag1wx,
                             c1pv[:, rows, 1:33], start=True, stop=False)
            nc.tensor.matmul(p_a1[:, hh * 512:(hh + 1) * 512], t_ag1wg,
                             u1pv[:, rows, 1:33], start=False, stop=True)
        d1 = sbw.tile([64, 1024], f32, tag="d1")
        p_d1 = pu.tile([64, 1024], f32, tag="pu")

        def dec1_taps(hh, wtile, srcv, start):
            cols = slice(hh * 512, (hh + 1) * 512)
            for tap in range(9):
                dy, dx = tap // 3, tap % 3
                rows = slice(dy + 16 * hh, dy + 16 * hh + 16)
                nc.tensor.matmul(p_d1[:, cols], wtile[:, tap, :],
                                 srcv[:, rows, dx:dx + 32],
                                 start=(start and tap == 0),
                                 stop=(not start and tap == 8))

        dec1_taps(0, t_dec1wa, u1pv, True)
        r1 = sbt.tile([64, 1024], bf16, tag="t")
        nc.scalar.activation(r1[:], p_a1[:], AF.Relu)
        p_g1 = pu.tile([1, 1024], f32, tag="pu")
        for hh in range(2):
            nc.tensor.matmul(p_g1[:, hh * 512:(hh + 1) * 512], t_ag1ps,
                             r1[:, hh * 512:(hh + 1) * 512])
        dec1_taps(1, t_dec1wa, u1pv, True)
        a1 = sbt.tile([1, 1024], bf16, tag="a1")
        nc.scalar.activation(a1[:], p_g1[:], AF.Sigmoid)
        p_a1b = pu.tile([64, 1024], f32, tag="pu")
        for hh in range(2):
            nc.tensor.matmul(p_a1b[:, hh * 512:(hh + 1) * 512], ones_bf[:, :64],
                             a1[:, hh * 512:(hh + 1) * 512])
        att1pv = att1p[:].rearrange("c (h w) -> c h w", h=34, w=34)
        nc.vector.tensor_mul(att1pv[:, 1:33, 1:33],
                             p_a1b[:].rearrange("c (h w) -> c h w", h=32, w=32),
                             c1pv[:, 1:33, 1:33])
        dec1_taps(0, t_dec1wb, att1pv, False)
        dec1_taps(1, t_dec1wb, att1pv, False)
        nc.scalar.activation(d1[:], p_d1[:], AF.Relu, bias=t_dec1b)

        # ---------------- d1 gather -> mov rows 0:64 ------------------------
        d1g = sbt.tile([64, NH], f32, tag="d1g")
        nc.gpsimd.ap_gather(d1g[:].rearrange("c (n o) -> c n o", o=1),
                            d1[:].rearrange("c (n o) -> c n o", o=1), t_pidx[:],
                            channels=64, num_elems=1024, d=1, num_idxs=NH)
        nc.vector.tensor_copy(mov_h[0:64, :], d1g[:])
        nc.vector.tensor_copy(mov_t[0:64, :], d1g[:])

        ew_cm.__exit__(None, None, None)
        pw_cm.__exit__(None, None, None)
        pu_cm.__exit__(None, None, None)

        # ---------------- pair features + decoder --------------------------
        hsT = sbw.tile([128, KD, NH], bf16, tag="hsT")
        tsT = sbw.tile([128, KD, NH], bf16, tag="tsT")
        ph_cm = tc.tile_pool(name="ph", bufs=4, space="PSUM")
        ph = ph_cm.__enter__()
        pd_cm = tc.tile_pool(name="pd", bufs=2, space="PSUM")
        pd = pd_cm.__enter__()
        po_cm = tc.tile_pool(name="po", bufs=1, space="PSUM")
        po = po_cm.__enter__()
        p_out = po.tile([2, NH], f32, tag="po")
        for k in range(KD):
            for (stk, mv, bp, dstT) in ((stk_h, mov_h, t_hbp, hsT),
                                        (stk_t, mov_t, t_tbp, tsT)):
                p_hs = ph.tile([128, NH], f32, tag="ph")
                nc.tensor.matmul(p_hs[:], stk[0:96, k, :], mv[:])
                nc.scalar.activation(dstT[:, k, :], p_hs[:],
                                     AF.Tanh, bias=bp[:, k:k + 1])
            for half, wd in ((0, t_wdecA), (1, t_wdecB)):
                p_u = pd.tile([128, NH], f32, tag="pd")
                nc.tensor.matmul(p_u[:], wd[:, k, :], tsT[:, k, :])
                v = sbt.tile([128, NH], bf16, tag="v")
                nc.vector.tensor_mul(v[:], p_u[:], hsT[:, k, :])
                nc.tensor.matmul(p_out[:], t_ssum[:, 2 * half:2 * half + 2], v[:],
                                 start=(k == 0 and half == 0),
                                 stop=(k == KD - 1 and half == 1))
        out_sb = sbt.tile([2, NH], f32, tag="out")
        nc.scalar.activation(out_sb[:], p_out[:], AF.Identity, bias=t_decb)
        nc.sync.dma_start(y[:], out_sb[:])
        po_cm.__exit__(None, None, None)
        pd_cm.__exit__(None, None, None)
        ph_cm.__exit__(None, None, None)

    nc.compile()
    return nc


def _wrap16(idx, n_slots):
    """int16 index layout for gpsimd gathers: wrapped in 16 partitions,
    replicated across the 8 gpsimd cores."""
    out = np.zeros((128, n_slots), np.int16)
    for j, v in enumerate(idx):
        out[np.arange(8) * 16 + j % 16, j // 16] = v
    return out


def _bf(a):
    import ml_dtypes
    return np.asarray(a, np.float32).astype(ml_dtypes.bfloat16)


def _fill(lay, ncols, rows, dtype, vals):
    out = np.zeros((rows, ncols), dtype=dtype)
    for name, arr in vals.items():
        r0, r, c0, c = lay[name]
        a = np.asarray(arr)
        if a.ndim != 2:
            a = a.reshape(r, c)
        out[r0:r0 + a.shape[0], c0:c0 + a.shape[1]] = a
    return out


def pack_inputs(inputs):
    import ml_dtypes
    bfd = ml_dtypes.bfloat16
    x = np.asarray(inputs["x"], np.float32)
    entity_pos = np.asarray(inputs["entity_pos"])
    hts = np.asarray(inputs["hts"])

    def W(name):
        return np.asarray(inputs[name], np.float32)

    head_w, tail_w = W("head_w"), W("tail_w")
    fin_w = W("fin_w").reshape(OUT_CH, 64)
    fin_b = W("fin_b")
    w2h_f = fin_w.T @ head_w[D:]          # [64, 768]
    w2t_f = fin_w.T @ tail_w[D:]
    hb_f = W("head_b") + fin_b @ head_w[D:]
    tb_f = W("tail_b") + fin_b @ tail_w[D:]

    a64 = _fill(LAY_A64, NC_A64, 64, bfd, {
        "ident": _bf(np.eye(NE)),
        "enc1w3": _bf(W("enc1_w").reshape(64, 3, 3).transpose(2, 1, 0).reshape(3, 192)),
        "enc2w": _bf(W("enc2_w").reshape(128, 64, 9).transpose(1, 2, 0).reshape(64, 1152)),
        "dec1wb": _bf(W("dec1_w").reshape(64, 192, 9).transpose(1, 2, 0)[128:].reshape(64, 576)),
        "ag1wx": _bf(W("ag1_wx").reshape(64, 64).T),
        "ag1ps": _bf(W("ag1_psi").reshape(1, 64).T),
    })
    bott_c = _fill(LAY_BOTT, NC_BOTT, 128, bfd, {
        "bott": _bf(W("bott_w").reshape(256, 128, 9).transpose(1, 2, 0).reshape(128, 2304)),
    })
    ag2_c = _fill(LAY_AG2, NC_AG2, 128, bfd, {
        "ag2wg": _bf(W("ag2_wg").reshape(128, 256).T.reshape(2, 128, 128)
                     .transpose(1, 0, 2).reshape(128, 256)),
        "ag2wx": _bf(W("ag2_wx").reshape(128, 128).T),
        "ag2ps": _bf(W("ag2_psi").reshape(1, 128).T),
    })
    d2w = W("dec2_w").reshape(128, 384, 9).transpose(1, 2, 0)\
        .reshape(3, 128, 9, 128).transpose(1, 0, 2, 3)       # [128, kc, 9, 128]
    dec2a_c = _fill(LAY_DEC2A, NC_DEC2A, 128, bfd,
                    {"dec2a": _bf(d2w[:, 0:2].reshape(128, 2304))})
    dec2b_c = _fill(LAY_DEC2B, NC_DEC2B, 128, bfd,
                    {"dec2b": _bf(d2w[:, 2].reshape(128, 1152))})
    b128 = _fill(LAY_B128, NC_B128, 128, bfd, {
        "dec1wa": _bf(W("dec1_w").reshape(64, 192, 9).transpose(1, 2, 0)[:128]
                      .reshape(128, 576)),
        "ag1wg": _bf(W("ag1_wg").reshape(64, 128).T),
    })
    w1h = _bf(head_w[:D].reshape(KD, 128, D).transpose(1, 0, 2).reshape(128, KD * D))
    w1t = _bf(tail_w[:D].reshape(KD, 128, D).transpose(1, 0, 2).reshape(128, KD * D))

    stk_h = np.zeros((128, KD * 128), np.float32)
    stk_h[0:64] = w2h_f.reshape(64, KD, 128).reshape(64, KD * 128)
    stk_t = np.zeros((128, KD * 128), np.float32)
    stk_t[0:64] = w2t_f.reshape(64, KD, 128).reshape(64, KD * 128)
    wdq = W("decoder_w").reshape(G, 64, 64, 2)        # [g, i, j, o]
    wdA = np.zeros((128, KD, 128), np.float32)
    wdB = np.zeros((128, KD, 128), np.float32)
    for k in range(KD):
        wdA[0:64, k, 0:64] = wdq[2 * k, :, :, 0].T
        wdA[64:128, k, 64:128] = wdq[2 * k + 1, :, :, 0].T
        wdB[0:64, k, 0:64] = wdq[2 * k, :, :, 1].T
        wdB[64:128, k, 64:128] = wdq[2 * k + 1, :, :, 1].T
    ssum = np.zeros((128, 4), np.float32)
    ssum[:, 0] = 1.0
    ssum[:, 3] = 1.0
    f_c = _fill(LAY_F, NC_F, 128, bfd, {
        "stk_h": _bf(stk_h),
        "stk_t": _bf(stk_t),
        "wdecA": _bf(wdA.reshape(128, KD * 128)),
        "wdecB": _bf(wdB.reshape(128, KD * 128)),
        "ssum": _bf(ssum),
    })

    mf = _fill(LAY_MF, NC_MF, 128, np.float32, {
        "enc1b": W("enc1_b").reshape(64, 1),
        "enc2b": W("enc2_b").reshape(128, 1),
        "bottb": W("bott_b").reshape(2, 128).T,
        "dec2b_b": W("dec2_b").reshape(128, 1),
        "dec1b": W("dec1_b").reshape(64, 1),
        "hbp": hb_f.reshape(KD, 128).T,
        "tbp": tb_f.reshape(KD, 128).T,
        "decb": W("decoder_b").reshape(2, 1),
        # emask filled per-core below
    })

    shared = dict(a64=a64, bott_c=bott_c, ag2_c=ag2_c, dec2a_c=dec2a_c,
                  dec2b_c=dec2b_c, b128=b128, w1h=w1h, w1t=w1t, f_c=f_c)

    in_maps = []
    for c in range(NCORES):
        b, h = c // 2, c % 2
        m = dict(shared)
        m["x_b"] = np.ascontiguousarray(x[b])
        start = entity_pos[b, :, 0].astype(np.int64)
        idx = np.minimum(start + 1, L - 1).astype(np.int16)
        m["eidx"] = _wrap16(np.tile(idx, 4), 8)
        mfc = mf.copy()
        r0, r, c0, cc = LAY_MF["emask"]
        mfc[r0:r0 + NE, c0] = (start + 1 < L).astype(np.float32)
        m["mf"] = mfc
        hi = hts[b, h * NH:(h + 1) * NH, 0].astype(np.int64)
        ti = hts[b, h * NH:(h + 1) * NH, 1].astype(np.int64)
        ohm = np.zeros((64, NH), np.float32)
        ohm[hi, np.arange(NH)] = 1.0
        ohm[32 + ti, np.arange(NH)] = 1.0
        m["ohm"] = ohm.astype(bfd)
        m["pidx"] = _wrap16((hi * NE + ti).astype(np.int16), NH // 16)
        in_maps.append(m)
    return in_maps


_NC_CACHE = None


def get_nc():
    global _NC_CACHE
    if _NC_CACHE is None:
        _NC_CACHE = build_nc()
    return _NC_CACHE


def kernel(**inputs):
    nc = get_nc()
    in_maps = pack_inputs(inputs)
    res = run_bass_kernel_spmd(nc, in_maps, core_ids=list(range(NCORES)))
    out = np.empty((B * P, 2), np.float32)
    for c in range(NCORES):
        b, h = c // 2, c % 2
        yc = res.results[c]["y"]                  # [2, NH]
        out[b * P + h * NH:b * P + (h + 1) * NH, :] = yc.T
    return out


# revision 42
# speedup vs baseline: 1.0041x; 1.0041x over previous
"""Trainium2 Bass kernel for nn_CoreferenceResolver (coref UNet + pair decoder).

Sharding: core c handles batch b=c//2 and pair-half h=c%2 (496 of 992 pairs).
The gather/cosine/UNet stages are replicated on the two cores sharing a batch;
the extractor linears and group-bilinear decoder are sharded over pairs.

v1 design notes (vs the f32r baseline):
- all weights/activations bf16 (halves DMA bytes, PE still 1 cycle/row)
- fin 1x1 conv + amap gathers + W2 matmuls folded into host-precomputed
  W2' = fin_w^T @ head_w[768:] and a single d1 gather
- extractor = one stacked K=96 matmul per (k, extractor):
  mov rows 0:64 d1[pairs], 64:96 host-computed one-hot(hi)/(ti)
- group-bilinear decoder via block-diagonal weights: one full-width
  multiply + one column-sum matmul per (k, output)
- enc1 via 3 column-shifted padded images (K=3 matmuls, 6 total)
- weights arrive as a few packed DMA chunks ordered just-in-time so the
  cos->image DMA never queues behind megabytes of weight traffic
- PE warmup chain holds the p-state ramp so real matmuls price at full speed
"""
import os
import sys

for _p in ("/opt/trn_rl_repo",):
    if os.path.isdir(_p) and _p not in sys.path:
        sys.path.insert(0, _p)

import numpy as np

import concourse.bass as bass
import concourse.tile as tile
from concourse import bacc, mybir
from concourse.bass_utils import run_bass_kernel_spmd

f32 = mybir.dt.float32
i16 = mybir.dt.int16
bf16 = mybir.dt.bfloat16
AF = mybir.ActivationFunctionType
OP = mybir.AluOpType

B, L, D, H = 4, 1024, 768, 12
NE, P = 32, 992
BLOCK = 64
G = D // BLOCK          # 12 groups
OUT_CH = 256
NCORES = 8
NH = P // 2             # 496 pairs per core
KD = D // 128           # 6 chunks of the D dim

# ---------------------------------------------------------------------------
# packed-chunk layouts (shapes only; shared by build_nc and pack_inputs)
# entries: name -> (row0, rows, col0, cols); each chunk = one DRAM tensor.
# ---------------------------------------------------------------------------


def _mklayout(rows, entries):
    lay, col = {}, 0
    for name, r0, r, c in entries:
        lay[name] = (r0, r, col, c)
        col += c
    return lay, col


LAY_A64, NC_A64 = _mklayout(64, [
    ("ident", 0, 32, 32),
    ("enc1w3", 0, 3, 192),          # [dx, dy*64+c]
    ("enc2w", 0, 64, 1152),         # [64, 9*128]
    ("dec1wb", 0, 64, 576),         # [64, 9*64]
    ("ag1wx", 0, 64, 64),
    ("ag1ps", 0, 64, 1),
])
LAY_BOTT, NC_BOTT = _mklayout(128, [("bott", 0, 128, 2304)])   # [128, 9*256]
LAY_AG2, NC_AG2 = _mklayout(128, [
    ("ag2wg", 0, 128, 256),         # [128, 2*128]
    ("ag2wx", 0, 128, 128),
    ("ag2ps", 0, 128, 1),
])
LAY_DEC2A, NC_DEC2A = _mklayout(128, [("dec2a", 0, 128, 2304)])  # kc 0:2
LAY_DEC2B, NC_DEC2B = _mklayout(128, [("dec2b", 0, 128, 1152)])  # kc 2
LAY_B128, NC_B128 = _mklayout(128, [
    ("dec1wa", 0, 128, 576),        # [128, 9*64]
    ("ag1wg", 0, 128, 64),
])
LAY_F, NC_F = _mklayout(128, [
    ("stk_h", 0, 128, 768),         # rows 0:64 W2h'; 64:96 <- EW1 (device)
    ("stk_t", 0, 128, 768),         # rows 0:64 W2t'; 64:96 <- EW1t (device)
    ("wdecA", 0, 128, 768),         # blockdiag per k-chunk, o=0
    ("wdecB", 0, 128, 768),         # blockdiag per k-chunk, o=1
    ("ssum", 0, 128, 4),            # [all-ones|0 ; 0|all-ones] selectors
    ("decbr", 0, 1, 2),             # decoder bias as a 1-row stationary
    ("onesr", 0, 1, 512),           # ones row (moving for the bias matmul)
])
LAY_MF, NC_MF = _mklayout(128, [
    ("emask", 0, 32, 1),
    ("enc1b", 0, 64, 1),
    ("enc2b", 0, 128, 1),
    ("bottb", 0, 128, 2),
    ("dec2b_b", 0, 128, 1),
    ("dec1b", 0, 64, 1),
    ("hbp", 0, 128, 6),
    ("tbp", 0, 128, 6),
    ("decb", 0, 2, 1),
])


def build_nc():
    nc = bacc.Bacc("TRN2", target_bir_lowering=False, debug=False, num_devices=NCORES)

    def inp(name, shape, dt=f32):
        return nc.dram_tensor(name, shape, dt, kind="ExternalInput")

    x_b = inp("x_b", [L, D])
    eidx_d = inp("eidx", [128, 8], i16)
    pidx_d = inp("pidx", [128, NH // 16], i16)
    ohm_d = inp("ohm", [64, NH], bf16)
    mf_d = inp("mf", [128, NC_MF])
    a64_d = inp("a64", [64, NC_A64], bf16)
    bott_d = inp("bott_c", [128, NC_BOTT], bf16)
    ag2_d = inp("ag2_c", [128, NC_AG2], bf16)
    dec2a_d = inp("dec2a_c", [128, NC_DEC2A], bf16)
    dec2b_d = inp("dec2b_c", [128, NC_DEC2B], bf16)
    b128_d = inp("b128", [128, NC_B128], bf16)
    w1h_d = inp("w1h", [128, KD * D], bf16)
    w1t_d = inp("w1t", [128, KD * D], bf16)
    f_d = inp("f_c", [128, NC_F], bf16)

    y = nc.dram_tensor("y", [2, NH], f32, kind="ExternalOutput")

    from contextlib import ExitStack
    with tile.TileContext(nc) as tc, ExitStack() as _ctx:
        sbw = _ctx.enter_context(tc.tile_pool(name="sbw", bufs=1))   # persistent
        sbt = _ctx.enter_context(tc.tile_pool(name="sbt", bufs=4))   # rotating temps
        pu_cm = tc.tile_pool(name="pu", bufs=2, space="PSUM")
        pu = pu_cm.__enter__()

        # ---------------- t0: gpsimd: eidx, gather, warm tile, memsets -----
        t_eidx = sbw.tile([128, 8], i16, tag="eidx")
        nc.sync.dma_start(t_eidx[:], eidx_d[:])
        warm = sbw.tile([1, 512], bf16, tag="warm")
        nc.vector.memset(warm[:], 0.0)
        # entities replicated on partition blocks 0:32 / 32:64 / 64:96 / 96:128
        ent_raw = sbw.tile([128, 1, D], f32, tag="entraw")
        nc.gpsimd.dma_gather(ent_raw[:], x_b[:], t_eidx[:],
                             num_idxs=128, num_idxs_reg=128, elem_size=D)
        ent = ent_raw[0:NE, 0, :]

        # padded intermediates (bf16) + border-only memsets
        img3 = sbw.tile([3, 34 * 34], bf16, tag="img3")
        c1p = sbw.tile([64, 34 * 34], bf16, tag="c1p")
        p1p = sbw.tile([64, 18 * 18], bf16, tag="p1p")
        c2p = sbw.tile([128, 18 * 18], bf16, tag="c2p")
        p2p = sbw.tile([128, 10 * 10], bf16, tag="p2p")
        u2p0 = sbw.tile([128, 18 * 18], bf16, tag="u2p0")
        u2p1 = sbw.tile([128, 18 * 18], bf16, tag="u2p1")
        att2p = sbw.tile([128, 18 * 18], bf16, tag="att2p")
        u1p = sbw.tile([128, 34 * 34], bf16, tag="u1p")
        att1p = sbw.tile([64, 34 * 34], bf16, tag="att1p")

        ones_bf = sbw.tile([1, 128], bf16, tag="ones")
        nc.vector.memset(ones_bf[:], 1.0)
        img3v0 = img3[:].rearrange("c (h w) -> c h w", h=34, w=34)
        nc.vector.memset(img3v0[:, 0:34:33, :], 0.0)
        nc.vector.memset(img3v0[:, :, 0:2], 0.0)
        nc.vector.memset(img3v0[:, :, 32:34], 0.0)

        def borders(t, n):
            v = t[:].rearrange("c (h w) -> c h w", h=n, w=n)
            nc.vector.memset(v[:, 0:n:n - 1, :], 0.0)
            nc.vector.memset(v[:, :, 0:n:n - 1], 0.0)

        for t, n in ((c1p, 34), (p1p, 18), (c2p, 18), (p2p, 10), (u2p0, 18),
                     (u2p1, 18), (att2p, 18), (u1p, 34), (att1p, 34)):
            borders(t, n)

        # ---------------- sync-engine DMA chunks (just-in-time order) ------
        def load(dram, shape, dt, tag, eng=None):
            t = sbw.tile(shape, dt, tag=tag)
            (eng or nc.sync).dma_start(t[:], dram[:])
            return t

        t_mf = load(mf_d, [128, NC_MF], f32, "mf")
        t_a64 = load(a64_d, [64, NC_A64], bf16, "a64")

        def loadE(dram, shape, dt, tag):
            t = sbw.tile(shape, dt, tag=tag)
            nc.vector.tensor_copy(t[0:1, 0:1], ent_raw[0:1, 0, 0:1])
            nc.sync.dma_start(t[:], dram[:])
            return t

        t_bott = loadE(bott_d, [128, NC_BOTT], bf16, "bott")
        t_ag2 = loadE(ag2_d, [128, NC_AG2], bf16, "ag2")
        t_pidx = load(pidx_d, [128, NH // 16], i16, "pidx")

        mov_h = sbw.tile([96, NH], bf16, tag="movh")
        mov_t = sbw.tile([96, NH], bf16, tag="movt")
        nc.sync.dma_start(mov_h[64:96, :], ohm_d[0:32, :])
        nc.sync.dma_start(mov_t[64:96, :], ohm_d[32:64, :])

        def vw(tile_, lay, name, shape=None):
            row0, r, c0, c = lay[name]
            ap = tile_[row0:row0 + r, c0:c0 + c]
            if shape is not None and len(shape) > 2:
                pat = {3: "p (a b) -> p a b", 4: "p (a b c) -> p a b c"}[len(shape)]
                kw = dict(zip("abc", shape[1:]))
                ap = ap.rearrange(pat, **kw)
            return ap

        t_ident = vw(t_a64, LAY_A64, "ident")
        t_enc1w = vw(t_a64, LAY_A64, "enc1w3", (3, 3, 64))
        t_enc2w = vw(t_a64, LAY_A64, "enc2w", (64, 9, 128))
        t_dec1wb = vw(t_a64, LAY_A64, "dec1wb", (64, 9, 64))
        t_ag1wx = vw(t_a64, LAY_A64, "ag1wx")
        t_ag1ps = vw(t_a64, LAY_A64, "ag1ps")
        t_bottw = vw(t_bott, LAY_BOTT, "bott", (128, 9, 256))
        t_ag2wg = vw(t_ag2, LAY_AG2, "ag2wg", (128, 2, 128))
        t_ag2wx = vw(t_ag2, LAY_AG2, "ag2wx")
        t_ag2ps = vw(t_ag2, LAY_AG2, "ag2ps")

        t_emask = vw(t_mf, LAY_MF, "emask")
        t_enc1b = vw(t_mf, LAY_MF, "enc1b")
        t_enc2b = vw(t_mf, LAY_MF, "enc2b")
        t_bottb = vw(t_mf, LAY_MF, "bottb")
        t_dec2bb = vw(t_mf, LAY_MF, "dec2b_b")
        t_dec1b = vw(t_mf, LAY_MF, "dec1b")
        t_hbp = vw(t_mf, LAY_MF, "hbp")
        t_tbp = vw(t_mf, LAY_MF, "tbp")
        t_decb = vw(t_mf, LAY_MF, "decb")

        # ---------------- PE warmup chain (p-state ramp) -------------------
        # keeps one unbroken PE busy-run so later bursts price at full speed
        pw_cm = tc.tile_pool(name="pw", bufs=1, space="PSUM")
        pw = pw_cm.__enter__()
        p_warm = pw.tile([1, 512], f32, tag="pw")

        def filler(n):
            for _ in range(n):
                nc.tensor.matmul(p_warm[:], warm[0:1, 0:1], warm[:],
                                 start=True, stop=True)

        filler(14)

        # ---------------- front-end: norms + transposes + cos --------------
        # dummy sqrt at t0 -> the preamble table load covers Sqrt+Square
        dummy = sbw.tile([1, 2], f32, tag="dum")
        nc.scalar.activation(dummy[:, 0:1], warm[0:1, 0:1], AF.Sqrt)
        sq_scr = sbt.tile([128, D], bf16, tag="t")
        ss = sbw.tile([128, 1], f32, tag="ss")
        nc.scalar.activation(sq_scr[0:96, :], ent_raw[0:96, 0, :], AF.Square,
                             accum_out=ss[0:96, :])
        normc = sbw.tile([128, 1], f32, tag="normc")
        nc.scalar.activation(normc[0:96, :], ss[0:96, :], AF.Sqrt)
        # dummy sigmoid -> hoist the 2nd act-table load off the critical path
        nc.scalar.activation(dummy[:, 1:2], ss[0:1, :], AF.Sigmoid)
        rinv = sbw.tile([NE, 1], f32, tag="rinv")
        nc.vector.reciprocal(rinv[:], normc[0:NE, :])
        nc.vector.tensor_tensor(out=rinv[:], in0=rinv[:], in1=t_emask, op=OP.mult)
        nrm = sbw.tile([NE, D], bf16, tag="nrm")
        nc.vector.tensor_scalar(out=nrm[:], in0=ent, scalar1=rinv[:],
                                scalar2=None, op0=OP.mult)

        nrmT = sbw.tile([128, KD, NE], bf16, tag="nrmT")
        p_T = pu.tile([128, KD * NE], bf16, tag="pu")
        for k in range(KD):
            nc.tensor.transpose(p_T[:, k * NE:(k + 1) * NE],
                                nrm[:, k * 128:(k + 1) * 128], t_ident)
        nc.vector.tensor_copy(nrmT[:], p_T[:].rearrange("p (a b) -> p a b", a=KD))

        p_cos = pu.tile([NE, NE], f32, tag="pu")
        for k in range(KD):
            nc.tensor.matmul(p_cos[:], nrmT[:, k, :], nrmT[:, k, :],
                             start=(k == 0), stop=(k == KD - 1))
        s_cos = sbw.tile([NE, NE], bf16, tag="scos")
        nc.vector.tensor_copy(s_cos[:], p_cos[:])
        filler(4)

        # ---------------- image staging: 3 column-shifted padded copies ----
        img3v = img3[:].rearrange("c (h w) -> c h w", h=34, w=34)
        nc.sync.dma_start(img3v[0:1, 1:33, 2:34], s_cos[:])
        nc.scalar.dma_start(img3v[1:2, 1:33, 1:33], s_cos[:])
        nc.gpsimd.dma_start(img3v[2:3, 1:33, 0:32], s_cos[:])

        # remaining weight chunks: a tiny token copy (reads s_cos) makes each
        # chunk DMA wait until the front-end is done with the DMA device
        def loadT(dram, shape, dt, tag):
            t = sbw.tile(shape, dt, tag=tag)
            nc.vector.tensor_copy(t[0:1, 0:1], s_cos[0:1, 0:1])
            nc.sync.dma_start(t[:], dram[:])
            return t

        t_w1h = loadT(w1h_d, [128, KD * D], bf16, "w1h")
        t_w1h = t_w1h[:].rearrange("p (k d) -> p k d", k=KD)
        t_w1t = loadT(w1t_d, [128, KD * D], bf16, "w1t")
        t_w1t = t_w1t[:].rearrange("p (k d) -> p k d", k=KD)
        t_dec2wa = loadT(dec2a_d, [128, NC_DEC2A], bf16, "dec2a")
        t_dec2wa = t_dec2wa[:].rearrange("p (a b c) -> p a b c", a=2, b=9, c=128)
        t_dec2wb = loadT(dec2b_d, [128, NC_DEC2B], bf16, "dec2b")
        t_dec2wb = t_dec2wb[:].rearrange("p (b c) -> p b c", b=9, c=128)
        t_b128 = loadT(b128_d, [128, NC_B128], bf16, "b128")
        t_dec1wa = vw(t_b128, LAY_B128, "dec1wa", (128, 9, 64))
        t_ag1wg = vw(t_b128, LAY_B128, "ag1wg")
        t_f = loadT(f_d, [128, NC_F], bf16, "f")
        stk_h = vw(t_f, LAY_F, "stk_h", (128, KD, 128))
        stk_t = vw(t_f, LAY_F, "stk_t", (128, KD, 128))
        t_wdecA = vw(t_f, LAY_F, "wdecA", (128, KD, 128))
        t_wdecB = vw(t_f, LAY_F, "wdecB", (128, KD, 128))
        t_ssum = vw(t_f, LAY_F, "ssum")
        t_decbr = vw(t_f, LAY_F, "decbr")
        t_onesr = vw(t_f, LAY_F, "onesr")

        # ---------------- enc1: 2 halves x 3 dy matmuls (K=3) --------------
        c1pv = c1p[:].rearrange("c (h w) -> c h w", h=34, w=34)
        p1pv = p1p[:].rearrange("c (h w) -> c h w", h=18, w=18)
        p_c1 = pu.tile([64, 1024], f32, tag="pu")
        p_c1v = p_c1[:].rearrange("c (h w) -> c h w", h=32, w=32)
        # enc1 in 4 row-quarter groups; pooling (on PSUM: maxpool commutes
        # with relu+per-channel-bias) chases each quarter
        tmp = sbt.tile([64, 16, 16], f32, tag="t")
        for q in range(4):
            for dy in range(3):
                rows = slice(dy + 8 * q, dy + 8 * q + 8)
                nc.tensor.matmul(p_c1[:, q * 256:(q + 1) * 256],
                                 t_enc1w[:, dy, :],
                                 img3v[0:3, rows, 1:33],
                                 start=(dy == 0), stop=(dy == 2))
            qr = slice(4 * q, 4 * q + 4)
            qv = p_c1v[:, 8 * q:8 * q + 8, :]
            nc.vector.tensor_copy(tmp[:, qr, :], qv[:, 0:8:2, 0:32:2])
            nc.vector.tensor_max(tmp[:, qr, :], tmp[:, qr, :], qv[:, 0:8:2, 1:32:2])
            nc.vector.tensor_max(tmp[:, qr, :], tmp[:, qr, :], qv[:, 1:8:2, 0:32:2])
            nc.vector.tensor_max(tmp[:, qr, :], tmp[:, qr, :], qv[:, 1:8:2, 1:32:2])
            nc.scalar.activation(p1pv[:, 1 + 4 * q:5 + 4 * q, 1:17], tmp[:, qr, :],
                                 AF.Relu, bias=t_enc1b)
        filler(7)
        nc.scalar.activation(c1pv[:, 1:33, 1:33], p_c1v[:],
                             AF.Relu, bias=t_enc1b)

        # ---------------- enc2: 9 taps K=64 --------------------------------
        p_c2 = pu.tile([128, 256], f32, tag="pu")
        for tap in range(9):
            dy, dx = tap // 3, tap % 3
            nc.tensor.matmul(p_c2[:], t_enc2w[:, tap, :],
                             p1pv[:, dy:dy + 16, dx:dx + 16],
                             start=(tap == 0), stop=(tap == 8))
        filler(4)
        c2pv = c2p[:].rearrange("c (h w) -> c h w", h=18, w=18)
        p2pv = p2p[:].rearrange("c (h w) -> c h w", h=10, w=10)
        p_c2v = p_c2[:].rearrange("c (h w) -> c h w", h=16, w=16)
        tmp2 = sbt.tile([128, 8, 8], f32, tag="t")
        nc.vector.tensor_copy(tmp2[:], p_c2v[:, 0:16:2, 0:16:2])
        nc.vector.tensor_max(tmp2[:], tmp2[:], p_c2v[:, 0:16:2, 1:16:2])
        nc.vector.tensor_max(tmp2[:], tmp2[:], p_c2v[:, 1:16:2, 0:16:2])
        nc.vector.tensor_max(tmp2[:], tmp2[:], p_c2v[:, 1:16:2, 1:16:2])
        nc.scalar.activation(p2pv[:, 1:9, 1:9], tmp2[:], AF.Relu, bias=t_enc2b)
        nc.scalar.activation(c2pv[:, 1:17, 1:17], p_c2v[:],
                             AF.Relu, bias=t_enc2b)

        # ---------------- EW-head premultiply (fills the pool stalls) ------
        ew_cm = tc.tile_pool(name="ew", bufs=1, space="PSUM")
        ew = ew_cm.__enter__()
        p_ewh = ew.tile([128, D], f32, tag="ew")
        for k in range(KD):
            for n0, n1 in ((0, 512), (512, D)):
                nc.tensor.matmul(p_ewh[64:96, n0:n1], nrmT[:, k, :],
                                 t_w1h[:, k, n0:n1],
                                 start=(k == 0), stop=(k == KD - 1))

        # EW-head PSUM -> stacked weights (frees the shared EW psum tile)
        nc.scalar.activation(stk_h[64:96, :, :].rearrange("p a b -> p (a b)"),
                             p_ewh[64:96, :], AF.Copy, scale=normc[64:96, :])

        # ---------------- bottleneck: 9 taps x 2 M-chunks, K=128 -----------
        c3 = []
        for mc in range(2):
            p_c3 = pu.tile([128, 64], f32, tag="pu")
            for tap in range(9):
                dy, dx = tap // 3, tap % 3
                nc.tensor.matmul(p_c3[:], t_bottw[:, tap, mc * 128:(mc + 1) * 128],
                                 p2pv[:, dy:dy + 8, dx:dx + 8],
                                 start=(tap == 0), stop=(tap == 8))
            c3s = sbt.tile([128, 8, 8], bf16, tag=f"c3_{mc}")
            nc.scalar.activation(c3s[:], p_c3[:].rearrange("c (h w) -> c h w", h=8, w=8),
                                 AF.Relu, bias=t_bottb[:, mc:mc + 1])
            c3.append(c3s)

        # ---------------- up2 ----------------------------------------------
        u2p0v = u2p0[:].rearrange("c (h w) -> c h w", h=18, w=18)
        u2p1v = u2p1[:].rearrange("c (h w) -> c h w", h=18, w=18)
        for src, dv in ((c3[0], u2p0v), (c3[1], u2p1v)):
            for i in range(2):
                for j in range(2):
                    nc.vector.tensor_copy(dv[:, 1 + i:17:2, 1 + j:17:2], src[:])

        # ---------------- attention gate 2 + dec2 (interleaved) ------------
        # the 18 u2-taps of dec2 fill the PE while the psi/sigmoid chain of
        # the attention gate bounces between ACT and DVE
        p_a2 = pu.tile([128, 256], f32, tag="pu")
        nc.tensor.matmul(p_a2[:], t_ag2wg[:, 0, :], u2p0v[:, 1:17, 1:17],
                         start=True, stop=False)
        nc.tensor.matmul(p_a2[:], t_ag2wg[:, 1, :], u2p1v[:, 1:17, 1:17],
                         start=False, stop=False)
        nc.tensor.matmul(p_a2[:], t_ag2wx, c2pv[:, 1:17, 1:17],
                         start=False, stop=True)
        p_d2 = pu.tile([128, 256], f32, tag="pu")
        n_mm = 0
        for kc in range(2):
            src = (u2p0v, u2p1v)[kc]
            for tap in range(9):
                dy, dx = tap // 3, tap % 3
                nc.tensor.matmul(p_d2[:], t_dec2wa[:, kc, tap, :],
                                 src[:, dy:dy + 16, dx:dx + 16],
                                 start=(n_mm == 0), stop=False)
                n_mm += 1
        p_ewt = ew.tile([128, D], f32, tag="ew")
        for k in range(KD):
            for n0, n1 in ((0, 512), (512, D)):
                nc.tensor.matmul(p_ewt[64:96, n0:n1], nrmT[:, k, :],
                                 t_w1t[:, k, n0:n1],
                                 start=(k == 0), stop=(k == KD - 1))
        r2 = sbt.tile([128, 256], bf16, tag="t")
        nc.scalar.activation(r2[:], p_a2[:], AF.Relu)
        p_g2 = pu.tile([1, 256], f32, tag="pu")
        nc.tensor.matmul(p_g2[:], t_ag2ps, r2[:])
        a2 = sbt.tile([1, 256], bf16, tag="a2")
        nc.scalar.activation(a2[:], p_g2[:], AF.Sigmoid)
        p_a2b = pu.tile([128, 256], f32, tag="pu")
        nc.tensor.matmul(p_a2b[:], ones_bf[:], a2[:])
        att2pv = att2p[:].rearrange("c (h w) -> c h w", h=18, w=18)
        nc.vector.tensor_mul(att2pv[:, 1:17, 1:17],
                             p_a2b[:].rearrange("c (h w) -> c h w", h=16, w=16),
                             c2pv[:, 1:17, 1:17])
        for tap in range(9):
            dy, dx = tap // 3, tap % 3
            nc.tensor.matmul(p_d2[:], t_dec2wb[:, tap, :],
                             att2pv[:, dy:dy + 16, dx:dx + 16],
                             start=False, stop=(tap == 8))
        d2s = sbt.tile([128, 256], bf16, tag="d2s")
        nc.scalar.activation(d2s[:], p_d2[:], AF.Relu, bias=t_dec2bb)
        nc.scalar.activation(stk_t[64:96, :, :].rearrange("p a b -> p (a b)"),
                             p_ewt[64:96, :], AF.Copy, scale=normc[64:96, :])

        # ---------------- up1 ----------------------------------------------
        u1pv = u1p[:].rearrange("c (h w) -> c h w", h=34, w=34)
        d2v = d2s[:].rearrange("c (h w) -> c h w", h=16, w=16)
        for i in range(2):
            for j in range(2):
                nc.vector.tensor_copy(u1pv[:, 1 + i:33:2, 1 + j:33:2], d2v[:])

        # ---------------- attention gate 1 + dec1 + EW (interleaved) -------
        # dec1's u1-taps and the EW premultiplies fill the PE while the
        # psi/sigmoid chain runs; att1-taps close the dec1 groups afterwards
        p_a1 = pu.tile([64, 1024], f32, tag="pu")
        for hh in range(2):
            rows = slice(1 + 16 * hh, 17 + 16 * hh)
            nc.tensor.matmul(p_a1[:, hh * 512:(hh + 1) * 512], t_ag1wx,
                             c1pv[:, rows, 1:33], start=True, stop=False)
            nc.tensor.matmul(p_a1[:, hh * 512:(hh + 1) * 512], t_ag1wg,
                             u1pv[:, rows, 1:33], start=False, stop=True)
        d1 = sbw.tile([64, 1024], f32, tag="d1")
        p_d1 = pu.tile([64, 1024], f32, tag="pu")

        def dec1_taps(hh, wtile, srcv, start):
            cols = slice(hh * 512, (hh + 1) * 512)
            for tap in range(9):
                dy, dx = tap // 3, tap % 3
                rows = slice(dy + 16 * hh, dy + 16 * hh + 16)
                nc.tensor.matmul(p_d1[:, cols], wtile[:, tap, :],
                                 srcv[:, rows, dx:dx + 32],
                                 start=(start and tap == 0),
                                 stop=(not start and tap == 8))

        dec1_taps(0, t_dec1wa, u1pv, True)
        r1 = sbt.tile([64, 1024], bf16, tag="t")
        nc.scalar.activation(r1[:], p_a1[:], AF.Relu)
        p_g1 = pu.tile([1, 1024], f32, tag="pu")
        for hh in range(2):
            nc.tensor.matmul(p_g1[:, hh * 512:(hh + 1) * 512], t_ag1ps,
                             r1[:, hh * 512:(hh + 1) * 512])
        dec1_taps(1, t_dec1wa, u1pv, True)
        a1 = sbt.tile([1, 1024], bf16, tag="a1")
        nc.scalar.activation(a1[:], p_g1[:], AF.Sigmoid)
        p_a1b = pu.tile([64, 1024], f32, tag="pu")
        for hh in range(2):
            nc.tensor.matmul(p_a1b[:, hh * 512:(hh + 1) * 512], ones_bf[:, :64],
                             a1[:, hh * 512:(hh + 1) * 512])
        att1pv = att1p[:].rearrange("c (h w) -> c h w", h=34, w=34)
        p_a1bv = p_a1b[:].rearrange("c (h w) -> c h w", h=32, w=32)
        nc.vector.tensor_mul(att1pv[:, 1:19, 1:33], p_a1bv[:, 0:18, :],
                             c1pv[:, 1:19, 1:33])
        nc.vector.tensor_mul(att1pv[:, 19:33, 1:33], p_a1bv[:, 18:32, :],
                             c1pv[:, 19:33, 1:33])
        dec1_taps(0, t_dec1wb, att1pv, False)
        dec1_taps(1, t_dec1wb, att1pv, False)
        nc.scalar.activation(d1[:], p_d1[:], AF.Relu, bias=t_dec1b)

        # ---------------- d1 gather -> mov rows 0:64 ------------------------
        d1g = sbt.tile([64, NH], f32, tag="d1g")
        nc.gpsimd.ap_gather(d1g[:].rearrange("c (n o) -> c n o", o=1),
                            d1[:].rearrange("c (n o) -> c n o", o=1), t_pidx[:],
                            channels=64, num_elems=1024, d=1, num_idxs=NH)
        nc.vector.tensor_copy(mov_h[0:64, :], d1g[:])
        nc.vector.tensor_copy(mov_t[0:64, :], d1g[:])

        ew_cm.__exit__(None, None, None)
        pw_cm.__exit__(None, None, None)
        pu_cm.__exit__(None, None, None)

        # ---------------- pair features + decoder --------------------------
        hsT = sbw.tile([128, KD, NH], bf16, tag="hsT")
        tsT = sbw.tile([128, KD, NH], bf16, tag="tsT")
        ph_cm = tc.tile_pool(name="ph", bufs=4, space="PSUM")
        ph = ph_cm.__enter__()
        pd_cm = tc.tile_pool(name="pd", bufs=3, space="PSUM")
        pd = pd_cm.__enter__()
        po_cm = tc.tile_pool(name="po", bufs=1, space="PSUM")
        po = po_cm.__enter__()
        p_out = po.tile([2, NH], f32, tag="po")
        nc.tensor.matmul(p_out[:], t_decbr, t_onesr[:, 0:NH],
                         start=True, stop=False)
        for k in range(KD):
            for (stk, mv, bp, dstT) in ((stk_h, mov_h, t_hbp, hsT),
                                        (stk_t, mov_t, t_tbp, tsT)):
                p_hs = ph.tile([128, NH], f32, tag="ph")
                nc.tensor.matmul(p_hs[:], stk[0:96, k, :], mv[:])
                nc.scalar.activation(dstT[:, k, :], p_hs[:],
                                     AF.Tanh, bias=bp[:, k:k + 1])
            for half, wd in ((0, t_wdecA), (1, t_wdecB)):
                p_u = pd.tile([128, NH], f32, tag="pd")
                nc.tensor.matmul(p_u[:], wd[:, k, :], tsT[:, k, :])
                v = sbt.tile([128, NH], bf16, tag="v")
                nc.vector.tensor_mul(v[:], p_u[:], hsT[:, k, :])
                nc.tensor.matmul(p_out[:], t_ssum[:, 2 * half:2 * half + 2], v[:],
                                 start=False,
                                 stop=(k == KD - 1 and half == 1))
        out_sb = sbt.tile([2, NH], f32, tag="out")
        nc.vector.tensor_copy(out_sb[:], p_out[:])
        nc.sync.dma_start(y[:], out_sb[:])
        po_cm.__exit__(None, None, None)
        pd_cm.__exit__(None, None, None)
        ph_cm.__exit__(None, None, None)

    nc.compile()
    return nc


def _wrap16(idx, n_slots):
    """int16 index layout for gpsimd gathers: wrapped in 16 partitions,
    replicated across the 8 gpsimd cores."""
    out = np.zeros((128, n_slots), np.int16)
    for j, v in enumerate(idx):
        out[np.arange(8) * 16 + j % 16, j // 16] = v
    return out


def _bf(a):
    import ml_dtypes
    return np.asarray(a, np.float32).astype(ml_dtypes.bfloat16)


def _fill(lay, ncols, rows, dtype, vals):
    out = np.zeros((rows, ncols), dtype=dtype)
    for name, arr in vals.items():
        r0, r, c0, c = lay[name]
        a = np.asarray(arr)
        if a.ndim != 2:
            a = a.reshape(r, c)
        out[r0:r0 + a.shape[0], c0:c0 + a.shape[1]] = a
    return out


def pack_inputs(inputs):
    import ml_dtypes
    bfd = ml_dtypes.bfloat16
    x = np.asarray(inputs["x"], np.float32)
    entity_pos = np.asarray(inputs["entity_pos"])
    hts = np.asarray(inputs["hts"])

    def W(name):
        return np.asarray(inputs[name], np.float32)

    head_w, tail_w = W("head_w"), W("tail_w")
    fin_w = W("fin_w").reshape(OUT_CH, 64)
    fin_b = W("fin_b")
    w2h_f = fin_w.T @ head_w[D:]          # [64, 768]
    w2t_f = fin_w.T @ tail_w[D:]
    hb_f = W("head_b") + fin_b @ head_w[D:]
    tb_f = W("tail_b") + fin_b @ tail_w[D:]

    a64 = _fill(LAY_A64, NC_A64, 64, bfd, {
        "ident": _bf(np.eye(NE)),
        "enc1w3": _bf(W("enc1_w").reshape(64, 3, 3).transpose(2, 1, 0).reshape(3, 192)),
        "enc2w": _bf(W("enc2_w").reshape(128, 64, 9).transpose(1, 2, 0).reshape(64, 1152)),
        "dec1wb": _bf(W("dec1_w").reshape(64, 192, 9).transpose(1, 2, 0)[128:].reshape(64, 576)),
        "ag1wx": _bf(W("ag1_wx").reshape(64, 64).T),
        "ag1ps": _bf(W("ag1_psi").reshape(1, 64).T),
    })
    bott_c = _fill(LAY_BOTT, NC_BOTT, 128, bfd, {
        "bott": _bf(W("bott_w").reshape(256, 128, 9).transpose(1, 2, 0).reshape(128, 2304)),
    })
    ag2_c = _fill(LAY_AG2, NC_AG2, 128, bfd, {
        "ag2wg": _bf(W("ag2_wg").reshape(128, 256).T.reshape(2, 128, 128)
                     .transpose(1, 0, 2).reshape(128, 256)),
        "ag2wx": _bf(W("ag2_wx").reshape(128, 128).T),
        "ag2ps": _bf(W("ag2_psi").reshape(1, 128).T),
    })
    d2w = W("dec2_w").reshape(128, 384, 9).transpose(1, 2, 0)\
        .reshape(3, 128, 9, 128).transpose(1, 0, 2, 3)       # [128, kc, 9, 128]
    dec2a_c = _fill(LAY_DEC2A, NC_DEC2A, 128, bfd,
                    {"dec2a": _bf(d2w[:, 0:2].reshape(128, 2304))})
    dec2b_c = _fill(LAY_DEC2B, NC_DEC2B, 128, bfd,
                    {"dec2b": _bf(d2w[:, 2].reshape(128, 1152))})
    b128 = _fill(LAY_B128, NC_B128, 128, bfd, {
        "dec1wa": _bf(W("dec1_w").reshape(64, 192, 9).transpose(1, 2, 0)[:128]
                      .reshape(128, 576)),
        "ag1wg": _bf(W("ag1_wg").reshape(64, 128).T),
    })
    w1h = _bf(head_w[:D].reshape(KD, 128, D).transpose(1, 0, 2).reshape(128, KD * D))
    w1t = _bf(tail_w[:D].reshape(KD, 128, D).transpose(1, 0, 2).reshape(128, KD * D))

    stk_h = np.zeros((128, KD * 128), np.float32)
    stk_h[0:64] = w2h_f.reshape(64, KD, 128).reshape(64, KD * 128)
    stk_t = np.zeros((128, KD * 128), np.float32)
    stk_t[0:64] = w2t_f.reshape(64, KD, 128).reshape(64, KD * 128)
    wdq = W("decoder_w").reshape(G, 64, 64, 2)        # [g, i, j, o]
    wdA = np.zeros((128, KD, 128), np.float32)
    wdB = np.zeros((128, KD, 128), np.float32)
    for k in range(KD):
        wdA[0:64, k, 0:64] = wdq[2 * k, :, :, 0].T
        wdA[64:128, k, 64:128] = wdq[2 * k + 1, :, :, 0].T
        wdB[0:64, k, 0:64] = wdq[2 * k, :, :, 1].T
        wdB[64:128, k, 64:128] = wdq[2 * k + 1, :, :, 1].T
    ssum = np.zeros((128, 4), np.float32)
    ssum[:, 0] = 1.0
    ssum[:, 3] = 1.0
    f_c = _fill(LAY_F, NC_F, 128, bfd, {
        "stk_h": _bf(stk_h),
        "stk_t": _bf(stk_t),
        "wdecA": _bf(wdA.reshape(128, KD * 128)),
        "wdecB": _bf(wdB.reshape(128, KD * 128)),
        "ssum": _bf(ssum),
        "decbr": _bf(W("decoder_b").reshape(1, 2)),
        "onesr": _bf(np.ones((1, 512))),
    })

    mf = _fill(LAY_MF, NC_MF, 128, np.float32, {
        "enc1b": W("enc1_b").reshape(64, 1),
        "enc2b": W("enc2_b").reshape(128, 1),
        "bottb": W("bott_b").reshape(2, 128).T,
        "dec2b_b": W("dec2_b").reshape(128, 1),
        "dec1b": W("dec1_b").reshape(64, 1),
        "hbp": hb_f.reshape(KD, 128).T,
        "tbp": tb_f.reshape(KD, 128).T,
        "decb": W("decoder_b").reshape(2, 1),
        # emask filled per-core below
    })

    shared = dict(a64=a64, bott_c=bott_c, ag2_c=ag2_c, dec2a_c=dec2a_c,
                  dec2b_c=dec2b_c, b128=b128, w1h=w1h, w1t=w1t, f_c=f_c)

    in_maps = []
    for c in range(NCORES):
        b, h = c // 2, c % 2
        m = dict(shared)
        m["x_b"] = np.ascontiguousarray(x[b])
        start = entity_pos[b, :, 0].astype(np.int64)
        idx = np.minimum(start + 1, L - 1).astype(np.int16)
        m["eidx"] = _wrap16(np.tile(idx, 4), 8)
        mfc = mf.copy()
        r0, r, c0, cc = LAY_MF["emask"]
        mfc[r0:r0 + NE, c0] = (start + 1 < L).astype(np.float32)
        m["mf"] = mfc
        hi = hts[b, h * NH:(h + 1) * NH, 0].astype(np.int64)
        ti = hts[b, h * NH:(h + 1) * NH, 1].astype(np.int64)
        ohm = np.zeros((64, NH), np.float32)
        ohm[hi, np.arange(NH)] = 1.0
        ohm[32 + ti, np.arange(NH)] = 1.0
        m["ohm"] = ohm.astype(bfd)
        m["pidx"] = _wrap16((hi * NE + ti).astype(np.int16), NH // 16)
        in_maps.append(m)
    return in_maps


_NC_CACHE = None


def get_nc():
    global _NC_CACHE
    if _NC_CACHE is None:
        _NC_CACHE = build_nc()
    return _NC_CACHE


def kernel(**inputs):
    nc = get_nc()
    in_maps = pack_inputs(inputs)
    res = run_bass_kernel_spmd(nc, in_maps, core_ids=list(range(NCORES)))
    out = np.empty((B * P, 2), np.float32)
    for c in range(NCORES):
        b, h = c // 2, c % 2
        yc = res.results[c]["y"]                  # [2, NH]
        out[b * P + h * NH:b * P + (h + 1) * NH, :] = yc.T
    return out
